# revision 52
# baseline (speedup 1.0000x reference)
"""Trainium2 Bass kernel for AdaptedEmbedding (embedding gather + LoRA).

out[b,s,:] = emb_weight[input[b,s], :] + (lora_A[:, input[b,s]].T @ lora_B.T) * (alpha/r)

Strategy (data-parallel over tokens, no collectives):
  Host:
    - Compact the vocab: only the unique-token rows of this batch
      (U <= 16384) are shipped, in bf16 (rel err ~2e-3, tolerance 2e-2).
    - bt[16, 1024] bf16 = (lora_B * scaling)^T, replicated.
    - Per core: 2048 tokens (contiguous shard), sorted by table row for
      HBM read locality; the tiny per-token A^T slice is shipped dense,
      pre-transposed, as at[16, 2048] bf16 -- the heavy gather
      (embedding rows, 4MB/core) runs on device.
  Device (per core):
    - 16 indirect DMA gathers (128 rows each) pull embedding rows,
      deep-buffered (12 tiles); the SWDGE drain ring is the pacer.
    - Per 128-token chunk: 2 bf16 matmuls (K=16) with bt into a per-chunk
      PSUM tile (4 buffers, so the matmul->add round-trip never stalls
      the gather-paced add stream), one 1024-wide vector add
      (gathered + lora) into a bf16 staging tile.
    - Per 256-token group: one 4KB-per-partition DMA writeback.
  Host: un-permute, upcast to f32 -> (4, 4096, 1024).
"""

import numpy as np

B, S = 4, 4096
DIM = 1024
R = 16
SCALING = 2.0
N_CORES = 8
TOK_PER_CORE = (B * S) // N_CORES  # 2048
P = 128
N_GROUPS = 8
GROUP_TOK = TOK_PER_CORE // N_GROUPS  # 256
CHUNKS_PER_GROUP = GROUP_TOK // P  # 2
N_CHUNKS = TOK_PER_CORE // P  # 16
EMB_BF16 = True  # bf16 embedding table: halves gather traffic


def _build_graph(u_rows: int):
    import concourse.bacc as bacc
    import concourse.bass as bass
    import concourse.mybir as mybir
    import concourse.tile as tile

    f32 = mybir.dt.float32
    bf16 = mybir.dt.bfloat16
    emb_dt = bf16 if EMB_BF16 else f32

    nc = bacc.Bacc("TRN2", target_bir_lowering=False)

    idx32 = nc.declare_dram_parameter("idx32", [P, N_CHUNKS], mybir.dt.int32, isOutput=False)
    emb = nc.declare_dram_parameter("emb", [u_rows, DIM], emb_dt, isOutput=False)
    at = nc.declare_dram_parameter("at", [R, TOK_PER_CORE], bf16, isOutput=False)
    bt = nc.declare_dram_parameter("bt", [R, DIM], bf16, isOutput=False)
    # permuted layout: [group, partition, chunk*dim] -- 4KB contiguous per
    # partition per write, host un-permutes
    out = nc.declare_dram_parameter(
        "out", [N_GROUPS, P, CHUNKS_PER_GROUP * DIM], bf16, isOutput=True
    )

    with tile.TileContext(nc) as tc:
        with (
            tc.tile_pool(name="persist", bufs=1) as pers,
            tc.tile_pool(name="sbuf", bufs=12) as sb,
            tc.tile_pool(name="outp", bufs=4) as op,
            tc.tile_pool(name="psum", bufs=4, space="PSUM") as ps,
        ):
            # dummy 16B/row gather absorbs SWDGE ring-init/dispatch latency
            # while the real idx tile is still in flight
            dummy_idx = pers.tile([P, 1], dtype=mybir.dt.int32)
            nc.gpsimd.memset(dummy_idx[:], 0)
            dummy_out = pers.tile([P, 8], dtype=emb_dt)
            nc.gpsimd.indirect_dma_start(
                out=dummy_out[:],
                out_offset=None,
                in_=emb[:],
                in_offset=bass.IndirectOffsetOnAxis(ap=dummy_idx[:], axis=0),
            )

            idx_sb = pers.tile([P, N_CHUNKS], dtype=mybir.dt.int32)
            nc.sync.dma_start(out=idx_sb[:, 0:1], in_=idx32[:, 0:1])
            nc.sync.dma_start(out=idx_sb[:, 1:], in_=idx32[:, 1:])
            bt_sb = pers.tile([R, DIM], dtype=bf16)
            nc.scalar.dma_start(out=bt_sb[:], in_=bt[:])
            a_t = pers.tile([R, TOK_PER_CORE], dtype=bf16)
            nc.scalar.dma_start(out=a_t[:], in_=at[:])

            for k in range(N_GROUPS):
                o = op.tile([P, CHUNKS_PER_GROUP * DIM], dtype=bf16, tag="o")
                for c in range(CHUNKS_PER_GROUP):
                    ch = k * CHUNKS_PER_GROUP + c
                    g = sb.tile([P, DIM], dtype=emb_dt, tag="g")
                    nc.gpsimd.indirect_dma_start(
                        out=g[:],
                        out_offset=None,
                        in_=emb[:],
                        in_offset=bass.IndirectOffsetOnAxis(
                            ap=idx_sb[:, ch : ch + 1], axis=0
                        ),
                    )
                    tok0 = ch * P
                    lora_ps = ps.tile([P, DIM], dtype=f32, tag="lora_ps")
                    for h in range(2):
                        nc.tensor.matmul(
                            out=lora_ps[:, h * 512 : (h + 1) * 512],
                            lhsT=a_t[:, tok0 : tok0 + P],
                            rhs=bt_sb[:, h * 512 : (h + 1) * 512],
                            start=True, stop=True,
                        )
                    nc.vector.tensor_add(
                        out=o[:, c * DIM : (c + 1) * DIM], in0=g[:], in1=lora_ps[:]
                    )
                nc.sync.dma_start(out=out[k], in_=o[:])

    nc.finalize()
    return nc


def kernel(input, emb_weight, lora_A, lora_B):
    import ml_dtypes
    from concourse.bass_utils import run_bass_kernel_spmd

    ids = np.asarray(input).astype(np.int64).reshape(-1)  # (16384,)
    emb_weight = np.asarray(emb_weight, dtype=np.float32)
    lora_A = np.asarray(lora_A, dtype=np.float32)
    lora_B = np.asarray(lora_B, dtype=np.float32)

    uniq, inv = np.unique(ids, return_inverse=True)
    u_rows = len(uniq)
    emb_small = np.ascontiguousarray(emb_weight[uniq])
    if EMB_BF16:
        emb_small = emb_small.astype(ml_dtypes.bfloat16)
    bt_host = np.ascontiguousarray((lora_B * SCALING).T).astype(ml_dtypes.bfloat16)

    in_maps = []
    perms = []
    for c in range(N_CORES):
        sl_orig = inv[c * TOK_PER_CORE : (c + 1) * TOK_PER_CORE]
        # sort tokens by table row: consecutive gather descriptors hit
        # adjacent HBM rows; host un-permutes the output
        perm = np.argsort(sl_orig, kind="stable")
        perms.append(perm)
        sl = sl_orig[perm]
        # int32 idx layout for indirect gathers: [partition, chunk]
        idx32_core = np.ascontiguousarray(sl.astype(np.int32).reshape(N_CHUNKS, P).T)
        at_core = np.ascontiguousarray(
            lora_A[:, ids[c * TOK_PER_CORE : (c + 1) * TOK_PER_CORE][perm]]
        ).astype(ml_dtypes.bfloat16)
        in_maps.append(
            {"idx32": idx32_core, "emb": emb_small, "at": at_core, "bt": bt_host}
        )

    nc = _build_graph(u_rows)
    res = None
    for attempt in range(3):
        try:
            res = run_bass_kernel_spmd(nc, in_maps, list(range(N_CORES)))
            break
        except Exception:
            # transient NRT exec-unit failures usually clear after a trivial
            # op touches the devices; cleanse and retry
            if attempt == 2:
                raise
            import time

            import jax

            try:
                x = jax.numpy.ones((8, 8))
                (x @ x).block_until_ready()
            except Exception:
                pass
            time.sleep(2.0)
    # un-permute: [group, p, (c d)] -> sorted token (group*256 + c*128 + p),
    # then invert the per-core sort
    cores = []
    for i in range(N_CORES):
        sorted_rows = (
            np.asarray(res.results[i]["out"])
            .astype(np.float32)
            .reshape(N_GROUPS, P, CHUNKS_PER_GROUP, DIM)
            .transpose(0, 2, 1, 3)
            .reshape(TOK_PER_CORE, DIM)
        )
        orig = np.empty_like(sorted_rows)
        orig[perms[i]] = sorted_rows
        cores.append(orig)
    return np.concatenate(cores, axis=0).reshape(B, S, DIM)
